# revision 3
# baseline (speedup 1.0000x reference)
"""Trainium2 Bass kernel for nn_Encoder_61753039782402 (HD-computing encoder).

Math: out[b,d] = sign( sum_f parity( sum_t L[q(b,t,f), d-t] + sum_t id[f, d-t] ) - 20.5 )
where q(b,t,f) = trunc(16*x[b,t,f] - 1) wrapped mod 16.

Implementation per core (D sharded 8 ways, 256 output columns each):
  - one-hot level masks OH_q[t,(b,f)] built on DVE via an exact floor trick
  - shifted-L "circulant band" tiles SL_q[u,d'] = L[q, d0+d'-127+u] gathered by
    overlapping strided DMA from a bf16 DRAM scratch (time axis reversed so all
    strides are positive; x is passed time-reversed to match)
  - PSUM-accumulated bf16 matmul chain: 16 level chunks + 2 triangular-constant
    chunks folding in the id window-sum
  - parity + grouped reduce over f + threshold on DVE, transpose via PE, DMA out
"""

import os
from contextlib import ExitStack

import numpy as np

import concourse.bass as bass
import concourse.bacc as bacc
import concourse.mybir as mybir
import concourse.tile as tile
from concourse.bass_utils import run_bass_kernel_spmd

B, T, F, Q, D = 8, 128, 40, 16, 2048
NCORE = 8
DS = D // NCORE  # 256 output columns per core
W = 384          # per-core window-slice width for lw / idt
BF = B * F       # 320
f32, bf16, i32 = mybir.dt.float32, mybir.dt.bfloat16, mybir.dt.int32
AL = mybir.AluOpType
TWO23 = float(2 ** 23)


def emit_kernel(nc, tc, ctx, xr, lw, idt, out):
    sb = ctx.enter_context(tc.tile_pool(name="sb", bufs=1))
    psp = ctx.enter_context(tc.tile_pool(name="psp", bufs=1, space=bass.MemorySpace.PSUM))
    drp = ctx.enter_context(tc.tile_pool(name="drp", bufs=1, space=bass.MemorySpace.DRAM))

    # ---- load x (already time-reversed on host) as [u, b, f]
    xt = sb.tile([T, B, F], f32, tag="xt")
    nc.sync.dma_start(out=xt[:], in_=xr.rearrange("b u f -> u b f"))
    xt2 = xt[:].rearrange("u b f -> u (b f)")  # [128, 320]

    # ---- L window slice: int32 -> bf16 -> DRAM scratch (for strided gathers)
    lwi = sb.tile([128, Q * W // 128], i32, tag="lwi")
    nc.sync.dma_start(
        out=lwi[:],
        in_=lw.rearrange("q w -> (q w)").rearrange("(p c) -> p c", p=128),
    )
    lwc = sb.tile([128, Q * W // 128], bf16, tag="lwc")
    nc.vector.tensor_copy(out=lwc[:], in_=lwi[:])
    lwb = drp.tile([Q, W], bf16, tag="lwb")
    nc.sync.dma_start(
        out=lwb[:].rearrange("q w -> (q w)").rearrange("(p c) -> p c", p=128),
        in_=lwc[:],
    )

    # ---- id window slice (transposed [src, f]) -> bf16, replicated over b
    idi = sb.tile([128, 3 * F], i32, tag="idi")
    for j in range(3):
        nc.sync.dma_start(out=idi[:, j * F:(j + 1) * F], in_=idt[j * 128:(j + 1) * 128, :])
    idb = sb.tile([128, 3 * F], bf16, tag="idb")
    nc.vector.tensor_copy(out=idb[:], in_=idi[:])
    idr = sb.tile([128, 3, B, F], bf16, tag="idr")
    nc.vector.tensor_copy(out=idr[:, :, 0, :], in_=idb[:].rearrange("p (j f) -> p j f", j=3))
    nc.vector.tensor_copy(out=idr[:, :, 1:2, :], in_=idr[:, :, 0:1, :])
    nc.vector.tensor_copy(out=idr[:, :, 2:4, :], in_=idr[:, :, 0:2, :])
    nc.vector.tensor_copy(out=idr[:, :, 4:8, :], in_=idr[:, :, 0:4, :])

    # ---- iota-derived constants: strict-lower / upper-incl triangles, identity
    iot = sb.tile([128, 128], i32, tag="iot")
    nc.gpsimd.iota(out=iot[:], pattern=[[-1, 128]], base=0, channel_multiplier=1)  # p - m
    ltri = sb.tile([128, 128], bf16, tag="ltri")
    nc.vector.tensor_single_scalar(out=ltri[:], in_=iot[:], scalar=0, op=AL.is_gt)   # m < p
    utri = sb.tile([128, 128], bf16, tag="utri")
    nc.vector.tensor_single_scalar(out=utri[:], in_=iot[:], scalar=0, op=AL.is_le)   # m >= p
    iden = sb.tile([128, 128], f32, tag="iden")
    nc.vector.tensor_single_scalar(out=iden[:], in_=iot[:], scalar=0, op=AL.is_equal)

    # ---- exact floor(16x) via round-to-nearest + fixup
    t1 = sb.tile([T, BF], f32, tag="t1")
    nc.vector.tensor_scalar(out=t1[:], in0=xt2, scalar1=16.0, scalar2=TWO23,
                            op0=AL.mult, op1=AL.add)
    t2 = sb.tile([T, BF], f32, tag="t2")
    nc.vector.tensor_single_scalar(out=t2[:], in_=t1[:], scalar=TWO23, op=AL.subtract)
    t3 = sb.tile([T, BF], f32, tag="t3")
    nc.vector.scalar_tensor_tensor(out=t3[:], in0=t2[:], scalar=0.0625, in1=xt2,
                                   op0=AL.mult, op1=AL.is_gt)
    uf = sb.tile([T, BF], f32, tag="uf")
    nc.vector.tensor_tensor(out=uf[:], in0=t2[:], in1=t3[:], op=AL.subtract)
    ub = sb.tile([T, BF], bf16, tag="ub")
    nc.vector.tensor_copy(out=ub[:], in_=uf[:])
    zm = sb.tile([T, BF], bf16, tag="zm")
    nc.vector.tensor_single_scalar(out=zm[:], in_=xt2, scalar=0.0, op=AL.is_equal)

    # ---- one-hot level masks (level q <=> u == q+1; u==0 -> level 0 unless x==0 -> 15)
    oh = []
    for q in range(Q):
        if q == 15:
            oh.append(zm)
            continue
        t = sb.tile([T, BF], bf16, tag=f"oh{q}")
        if q == 0:
            nc.vector.scalar_tensor_tensor(out=t[:], in0=ub[:], scalar=1.0, in1=zm[:],
                                           op0=AL.is_le, op1=AL.subtract)
        else:
            nc.vector.tensor_single_scalar(out=t[:], in_=ub[:], scalar=float(q + 1),
                                           op=AL.is_equal)
        oh.append(t)

    # ---- shifted-L gathers: sl[q][u, d'] = Lw[q, u + d']  (overlapping strided DMA)
    sl = []
    for q in range(Q):
        t = sb.tile([128, DS], bf16, tag=f"sl{q}")
        src = bass.AP(tensor=lwb[:].tensor, offset=q * W, ap=[[1, 128], [1, DS]])
        nc.sync.dma_start(out=t[:], in_=src)
        sl.append(t)

    # ---- matmul chains (one PSUM bank per 128-wide output chunk)
    accs = []
    for mc in range(2):
        p = psp.tile([128, BF], f32, tag=f"acc{mc}")
        for qi in range(Q):
            nc.tensor.matmul(p[:], sl[qi][:, mc * 128:(mc + 1) * 128], oh[qi][:],
                             start=(qi == 0), stop=False)
        j_lo, j_hi = (0, 1) if mc == 0 else (1, 2)
        nc.tensor.matmul(p[:], ltri[:], idr[:, j_lo], start=False, stop=False)
        nc.tensor.matmul(p[:], utri[:], idr[:, j_hi], start=False, stop=True)
        accs.append(p)

    # ---- parity, grouped reduce over f, threshold
    fin = sb.tile([128, 2 * B], f32, tag="fin")
    for mc in range(2):
        si = sb.tile([128, BF], i32, tag=f"si{mc}")
        nc.vector.tensor_copy(out=si[:], in_=accs[mc][:])
        sj = sb.tile([128, BF], i32, tag=f"sj{mc}")
        nc.vector.tensor_single_scalar(out=sj[:], in_=si[:], scalar=1, op=AL.bitwise_and)
        red = sb.tile([128, B], i32, tag=f"red{mc}")
        with nc.allow_low_precision(reason="exact int32 accumulation of 40 0/1 values"):
            nc.vector.tensor_reduce(out=red[:], in_=sj[:].rearrange("p (b f) -> p b f", b=B),
                                    axis=mybir.AxisListType.X, op=AL.add)
        cmpo = sb.tile([128, B], f32, tag=f"cmp{mc}")
        nc.vector.tensor_single_scalar(out=cmpo[:], in_=red[:], scalar=20, op=AL.is_gt)
        nc.vector.tensor_scalar(out=fin[:, mc * B:(mc + 1) * B], in0=cmpo[:],
                                scalar1=2.0, scalar2=-1.0, op0=AL.mult, op1=AL.add)

    # ---- transpose [128, 16] -> [16, 128] on PE, then contiguous DMA out
    pst = psp.tile([2 * B, 128], f32, tag="pst")
    nc.tensor.transpose(out=pst[:], in_=fin[:], identity=iden[:])
    so = sb.tile([2 * B, 128], f32, tag="so")
    nc.vector.tensor_copy(out=so[:], in_=pst[:])
    for mc in range(2):
        nc.sync.dma_start(out=out[:, mc * 128:(mc + 1) * 128],
                          in_=so[mc * B:(mc + 1) * B, :])


def build_nc():
    nc = bacc.Bacc("TRN2", target_bir_lowering=False, debug=False)
    xr = nc.dram_tensor("xr", [B, T, F], f32, kind="ExternalInput")
    lw = nc.dram_tensor("lw", [Q, W], i32, kind="ExternalInput")
    idt = nc.dram_tensor("idt", [W, F], i32, kind="ExternalInput")
    out = nc.dram_tensor("out", [B, DS], f32, kind="ExternalOutput")
    with tile.TileContext(nc) as tc:
        with ExitStack() as ctx:
            emit_kernel(nc, tc, ctx, xr[:], lw[:], idt[:], out[:])
    nc.compile()
    return nc


def make_in_maps(x, level_hvs, id_hvs):
    x = np.asarray(x, dtype=np.float32)
    L = np.asarray(level_hvs, dtype=np.int32)
    ID = np.asarray(id_hvs, dtype=np.int32)
    xr = np.ascontiguousarray(x[:, ::-1, :])
    LL2 = np.concatenate([L, L], axis=1)
    II2 = np.concatenate([ID, ID], axis=1)
    in_maps = []
    for c in range(NCORE):
        d0 = c * DS
        s = (d0 - 127) % D
        lw_c = np.ascontiguousarray(LL2[:, s:s + W])
        s2 = (d0 - 128) % D
        idt_c = np.ascontiguousarray(II2[:, s2:s2 + W].T)
        in_maps.append({"xr": xr, "lw": lw_c, "idt": idt_c})
    return in_maps


_NC_CACHE = {}


def kernel(x, level_hvs, id_hvs):
    if "nc" not in _NC_CACHE:
        _NC_CACHE["nc"] = build_nc()
    nc = _NC_CACHE["nc"]
    in_maps = make_in_maps(x, level_hvs, id_hvs)
    res = run_bass_kernel_spmd(nc, in_maps, list(range(NCORE)))
    full = np.empty((B, D), dtype=np.float32)
    for c in range(NCORE):
        full[:, c * DS:(c + 1) * DS] = res.results[c]["out"]
    return full


# revision 12
# speedup vs baseline: 1.4406x; 1.4406x over previous
"""Trainium2 Bass kernel for nn_Encoder_61753039782402 (HD-computing encoder).

Math: out[b,d] = sign( sum_f parity( sum_t L[q(b,t,f), d-t] + sum_t id[f, d-t] ) - 20.5 )
where q(b,t,f) = trunc(16*x[b,t,f] - 1) wrapped mod 16 (x==0 -> 15).

Implementation per core (D sharded 8 ways, 256 output columns each):
  - one-hot level masks OH_q[t,(b,f)] built via an exact floor trick; the 15
    equality compares are split across DVE and GPSIMD
  - shifted-L "circulant band" tiles SL_q[u,d'] = L[q, d0+d'-127+u] gathered by
    two overlapping strided DMAs from the bf16 L window input (time axis
    reversed so all strides are positive; x is passed time-reversed)
  - PSUM-accumulated bf16 matmul chain: 16 level chunks + 2 triangular-constant
    chunks folding in the id window-sum
  - parity (mod 2) + grouped reduce over f + threshold on DVE; PE transpose;
    direct DMA out of PSUM
Host-side prep is layout/dtype only: slicing the doubled tables per core,
time-reversing/transposing x, int->bf16 casts.
"""

from contextlib import ExitStack

import numpy as np
import ml_dtypes

import concourse.bass as bass
import concourse.bacc as bacc
import concourse.mybir as mybir
import concourse.tile as tile
from concourse.bass_utils import run_bass_kernel_spmd

B, T, F, Q, D = 8, 128, 40, 16, 2048
NCORE = 8
DS = D // NCORE  # 256 output columns per core
W = 384          # per-core window-slice width for lwb / idt
BF = B * F       # 320
f32, bf16, i32 = mybir.dt.float32, mybir.dt.bfloat16, mybir.dt.int32
AL = mybir.AluOpType
TWO23 = float(2 ** 23)

PARITY_MODE = "int"   # "mod" (single fused fp-mod op) fails walrus tensor_scalar_valid_ops
N_POOL_CMP = 7        # of the 14 plain equality compares, how many go to GPSIMD


def emit_kernel(nc, tc, ctx, xt_d, lwb_d, idt_d, out_d):
    sb = ctx.enter_context(tc.tile_pool(name="sb", bufs=1))
    psp = ctx.enter_context(tc.tile_pool(name="psp", bufs=1, space=bass.MemorySpace.PSUM))

    # ---- input DMAs ------------------------------------------------------
    xt = sb.tile([T, B, F], f32, tag="xt")
    nc.sync.dma_start(out=xt[:], in_=xt_d)
    xt2 = xt[:].rearrange("u b f -> u (b f)")  # [128, 320]

    # shifted-L gathers: sl[u, q, d'] = Lw[q, u + d']   (overlapping reads)
    # issued from ACT's HWDGE path to overlap with SP's x trigger
    sla = sb.tile([128, Q, DS], bf16, tag="sla")
    for g in range(4):
        src = bass.AP(tensor=lwb_d.tensor, offset=g * 4 * W,
                      ap=[[1, 128], [W, 4], [1, DS]])
        nc.scalar.dma_start(out=sla[:, g * 4:(g + 1) * 4, :], in_=src)

    # id window slice, transposed [src, f]: one 3-chunk DMA via Pool SWDGE
    idb = sb.tile([128, 3, F], bf16, tag="idb")
    nc.gpsimd.dma_start(out=idb[:], in_=idt_d.rearrange("(j p) f -> p j f", p=128))

    # ---- GPSIMD-side constants ------------------------------------------
    iot = sb.tile([128, 128], i32, tag="iot")
    nc.gpsimd.iota(out=iot[:], pattern=[[-1, 128]], base=0, channel_multiplier=1)  # p - m
    ltri = sb.tile([128, 128], bf16, tag="ltri")
    nc.gpsimd.tensor_single_scalar(out=ltri[:], in_=iot[:], scalar=0, op=AL.is_gt)   # m < p
    utri = sb.tile([128, 128], bf16, tag="utri")
    nc.gpsimd.tensor_single_scalar(out=utri[:], in_=iot[:], scalar=0, op=AL.is_le)   # m >= p
    iden = sb.tile([128, 128], f32, tag="iden")
    nc.gpsimd.tensor_single_scalar(out=iden[:], in_=iot[:], scalar=0, op=AL.is_equal)

    # replicate id window over b (log-doubling) on GPSIMD
    idr = sb.tile([128, 3, B, F], bf16, tag="idr")
    nc.gpsimd.tensor_copy(out=idr[:, :, 0, :], in_=idb[:])
    nc.gpsimd.tensor_copy(out=idr[:, :, 1:2, :], in_=idr[:, :, 0:1, :])
    nc.gpsimd.tensor_copy(out=idr[:, :, 2:4, :], in_=idr[:, :, 0:2, :])
    nc.gpsimd.tensor_copy(out=idr[:, :, 4:8, :], in_=idr[:, :, 0:4, :])

    # ---- exact floor(16x) via round-to-nearest + fixup (DVE) ------------
    t1 = sb.tile([T, BF], f32, tag="t1")
    nc.vector.tensor_scalar(out=t1[:], in0=xt2, scalar1=16.0, scalar2=TWO23,
                            op0=AL.mult, op1=AL.add)
    t2 = sb.tile([T, BF], f32, tag="t2")
    nc.vector.tensor_single_scalar(out=t2[:], in_=t1[:], scalar=TWO23, op=AL.subtract)
    t3 = sb.tile([T, BF], f32, tag="t3")
    nc.vector.scalar_tensor_tensor(out=t3[:], in0=t2[:], scalar=0.0625, in1=xt2,
                                   op0=AL.mult, op1=AL.is_gt)
    ub = sb.tile([T, BF], bf16, tag="ub")
    nc.vector.tensor_tensor(out=ub[:], in0=t2[:], in1=t3[:], op=AL.subtract)

    # ---- one-hot level masks --------------------------------------------
    # level q <=> u == q+1 for q in 1..14; q0 <=> u<=1 minus the x==0 case;
    # q15 <=> x == 0. Plain equality masks first (they gate the matmuls),
    # zero-mask and fused q0 afterwards.
    oh = [None] * Q
    for q in range(1, Q - 1):
        t = sb.tile([T, BF], bf16, tag=f"oh{q}")
        eng = nc.gpsimd if q >= Q - 1 - N_POOL_CMP else nc.vector
        eng.tensor_single_scalar(out=t[:], in_=ub[:], scalar=float(q + 1),
                                 op=AL.is_equal)
        oh[q] = t
    zm = sb.tile([T, BF], bf16, tag="zm")
    nc.vector.tensor_single_scalar(out=zm[:], in_=xt2, scalar=0.0, op=AL.is_equal)
    oh[Q - 1] = zm
    t0 = sb.tile([T, BF], bf16, tag="oh0")
    nc.vector.scalar_tensor_tensor(out=t0[:], in0=ub[:], scalar=1.0, in1=zm[:],
                                   op0=AL.is_le, op1=AL.subtract)
    oh[0] = t0

    # ---- matmul chains ---------------------------------------------------
    # accumulation order chosen by expected mask readiness: the plain equality
    # masks land first (q1..14), then the fused q0 / zero-mask q15, then the
    # id-window band terms
    chain = list(range(1, Q - 1)) + [0, Q - 1]
    accs = []
    for mc in range(2):
        p = psp.tile([128, BF], f32, tag=f"acc{mc}")
        for ci, qi in enumerate(chain):
            nc.tensor.matmul(p[:], sla[:, qi, mc * 128:(mc + 1) * 128], oh[qi][:],
                             start=(ci == 0), stop=False)
        j_lo, j_hi = (0, 1) if mc == 0 else (1, 2)
        nc.tensor.matmul(p[:], ltri[:], idr[:, j_lo], start=False, stop=False)
        nc.tensor.matmul(p[:], utri[:], idr[:, j_hi], start=False, stop=True)
        accs.append(p)

    # ---- parity, grouped reduce over f, threshold ------------------------
    fin = sb.tile([128, 2 * B], f32, tag="fin")
    for mc in range(2):
        if PARITY_MODE == "mod":
            seq = sb.tile([128, BF], bf16, tag=f"seq{mc}")
            nc.vector.tensor_single_scalar(out=seq[:], in_=accs[mc][:], scalar=2.0,
                                           op=AL.mod)
            thr = 20.5
        else:
            si = sb.tile([128, BF], i32, tag=f"si{mc}")
            nc.vector.tensor_copy(out=si[:], in_=accs[mc][:])
            seq = sb.tile([128, BF], i32, tag=f"seq{mc}")
            nc.vector.tensor_single_scalar(out=seq[:], in_=si[:], scalar=1,
                                           op=AL.bitwise_and)
            thr = 20
        red = sb.tile([128, B], seq[:].dtype, tag=f"red{mc}")
        with nc.allow_low_precision(reason="exact small-int accumulation (<=40)"):
            nc.vector.tensor_reduce(out=red[:], in_=seq[:].rearrange("p (b f) -> p b f", b=B),
                                    axis=mybir.AxisListType.X, op=AL.add)
        cmpo = sb.tile([128, B], f32, tag=f"cmp{mc}")
        nc.vector.tensor_single_scalar(out=cmpo[:], in_=red[:], scalar=thr, op=AL.is_gt)
        nc.vector.tensor_scalar(out=fin[:, mc * B:(mc + 1) * B], in0=cmpo[:],
                                scalar1=2.0, scalar2=-1.0, op0=AL.mult, op1=AL.add)

    # ---- transpose [128, 16] -> [16, 128] on PE, DMA straight from PSUM --
    pst = psp.tile([2 * B, 128], f32, tag="pst")
    nc.tensor.transpose(out=pst[:], in_=fin[:], identity=iden[:])
    so = sb.tile([2 * B, 128], f32, tag="so")
    nc.vector.tensor_copy(out=so[:], in_=pst[:])
    nc.sync.dma_start(out=out_d[:, 0:128], in_=so[0:B, :])
    nc.gpsimd.dma_start(out=out_d[:, 128:256], in_=so[B:2 * B, :])


def build_nc():
    nc = bacc.Bacc("TRN2", target_bir_lowering=False, debug=False)
    xt_d = nc.dram_tensor("xt", [T, B, F], f32, kind="ExternalInput")
    lwb_d = nc.dram_tensor("lwb", [Q, W], bf16, kind="ExternalInput")
    idt_d = nc.dram_tensor("idt", [W, F], bf16, kind="ExternalInput")
    out_d = nc.dram_tensor("out", [B, DS], f32, kind="ExternalOutput")
    with tile.TileContext(nc) as tc:
        with ExitStack() as ctx:
            emit_kernel(nc, tc, ctx, xt_d[:], lwb_d[:], idt_d[:], out_d[:])
    nc.compile()
    return nc


def make_in_maps(x, level_hvs, id_hvs):
    x = np.asarray(x, dtype=np.float32)
    L = np.asarray(level_hvs, dtype=np.int32)
    ID = np.asarray(id_hvs, dtype=np.int32)
    # time-reverse + transpose to [T, B, F] (layout only)
    xt = np.ascontiguousarray(x[:, ::-1, :].transpose(1, 0, 2))
    LL2 = np.concatenate([L, L], axis=1).astype(ml_dtypes.bfloat16)
    II2 = np.concatenate([ID, ID], axis=1).astype(ml_dtypes.bfloat16)
    in_maps = []
    for c in range(NCORE):
        d0 = c * DS
        s = (d0 - 127) % D
        lwb_c = np.ascontiguousarray(LL2[:, s:s + W])
        s2 = (d0 - 128) % D
        idt_c = np.ascontiguousarray(II2[:, s2:s2 + W].T)
        in_maps.append({"xt": xt, "lwb": lwb_c, "idt": idt_c})
    return in_maps


_NC_CACHE = {}


def kernel(x, level_hvs, id_hvs):
    if "nc" not in _NC_CACHE:
        _NC_CACHE["nc"] = build_nc()
    nc = _NC_CACHE["nc"]
    in_maps = make_in_maps(x, level_hvs, id_hvs)
    res = run_bass_kernel_spmd(nc, in_maps, list(range(NCORE)))
    full = np.empty((B, D), dtype=np.float32)
    for c in range(NCORE):
        full[:, c * DS:(c + 1) * DS] = res.results[c]["out"]
    return full


# revision 14
# speedup vs baseline: 1.5609x; 1.0835x over previous
"""Trainium2 Bass kernel for nn_Encoder_61753039782402 (HD-computing encoder).

Math: out[b,d] = sign( sum_f parity( sum_t L[q(b,t,f), d-t] + sum_t id[f, d-t] ) - 20.5 )
where q(b,t,f) = trunc(16*x[b,t,f] - 1) wrapped mod 16 (x==0 -> 15).

Implementation per core (D sharded 8 ways, 256 output columns each):
  - one-hot level masks OH_q[t,(b,f)] built via an exact floor trick; the 15
    equality compares are split across DVE and GPSIMD
  - shifted-L "circulant band" tiles SL_q[u,d'] = L[q, d0+d'-127+u] gathered by
    two overlapping strided DMAs from the bf16 L window input (time axis
    reversed so all strides are positive; x is passed time-reversed)
  - PSUM-accumulated bf16 matmul chain: 16 level chunks + 2 triangular-constant
    chunks folding in the id window-sum
  - parity (mod 2) + grouped reduce over f + threshold on DVE; PE transpose;
    direct DMA out of PSUM
Host-side prep is layout/dtype only: slicing the doubled tables per core,
time-reversing/transposing x, int->bf16 casts.
"""

from contextlib import ExitStack

import numpy as np
import ml_dtypes

import concourse.bass as bass
import concourse.bacc as bacc
import concourse.mybir as mybir
import concourse.tile as tile
from concourse.bass_utils import run_bass_kernel_spmd

B, T, F, Q, D = 8, 128, 40, 16, 2048
NCORE = 8
DS = D // NCORE  # 256 output columns per core
W = 384          # per-core window-slice width for lwb / idt
BF = B * F       # 320
f32, bf16, i32 = mybir.dt.float32, mybir.dt.bfloat16, mybir.dt.int32
f8 = mybir.dt.float8e4
AL = mybir.AluOpType
TWO23 = float(2 ** 23)

PARITY_MODE = "int"   # "mod" (single fused fp-mod op) fails walrus tensor_scalar_valid_ops
N_POOL_CMP = 7        # of the 14 plain equality compares, how many go to GPSIMD


def emit_kernel(nc, tc, ctx, xt_d, lwb_d, idt_d, out_d):
    sb = ctx.enter_context(tc.tile_pool(name="sb", bufs=1))
    psp = ctx.enter_context(tc.tile_pool(name="psp", bufs=1, space=bass.MemorySpace.PSUM))

    # ---- input DMAs ------------------------------------------------------
    xt = sb.tile([T, B, F], f32, tag="xt")
    nc.sync.dma_start(out=xt[:], in_=xt_d)
    xt2 = xt[:].rearrange("u b f -> u (b f)")  # [128, 320]

    # shifted-L gathers: sl[u, q, d'] = Lw[q, u + d']   (overlapping reads)
    # issued from ACT's HWDGE path to overlap with SP's x trigger
    sla = sb.tile([128, Q, DS], f8, tag="sla")
    for g in range(4):
        src = bass.AP(tensor=lwb_d.tensor, offset=g * 4 * W,
                      ap=[[1, 128], [W, 4], [1, DS]])
        nc.scalar.dma_start(out=sla[:, g * 4:(g + 1) * 4, :], in_=src)

    # id window slice, transposed [src, f]: one 3-chunk DMA via Pool SWDGE
    idb = sb.tile([128, 3, F], f8, tag="idb")
    nc.gpsimd.dma_start(out=idb[:], in_=idt_d.rearrange("(j p) f -> p j f", p=128))

    # ---- GPSIMD-side constants ------------------------------------------
    iot = sb.tile([128, 128], i32, tag="iot")
    nc.gpsimd.iota(out=iot[:], pattern=[[-1, 128]], base=0, channel_multiplier=1)  # p - m
    tri = sb.tile([128, 2, 128], f8, tag="tri")
    nc.gpsimd.tensor_single_scalar(out=tri[:, 0, :], in_=iot[:], scalar=0, op=AL.is_gt)  # m < p
    nc.gpsimd.tensor_single_scalar(out=tri[:, 1, :], in_=iot[:], scalar=0, op=AL.is_le)  # m >= p
    iden = sb.tile([128, 128], f32, tag="iden")
    nc.gpsimd.tensor_single_scalar(out=iden[:], in_=iot[:], scalar=0, op=AL.is_equal)

    # replicate id window over b (log-doubling) on GPSIMD
    idr = sb.tile([128, 3, B, F], f8, tag="idr")
    nc.gpsimd.tensor_copy(out=idr[:, :, 0, :], in_=idb[:])
    nc.gpsimd.tensor_copy(out=idr[:, :, 1:2, :], in_=idr[:, :, 0:1, :])
    nc.gpsimd.tensor_copy(out=idr[:, :, 2:4, :], in_=idr[:, :, 0:2, :])
    nc.gpsimd.tensor_copy(out=idr[:, :, 4:8, :], in_=idr[:, :, 0:4, :])

    # ---- exact floor(16x) via round-to-nearest + fixup (DVE) ------------
    t1 = sb.tile([T, BF], f32, tag="t1")
    nc.vector.tensor_scalar(out=t1[:], in0=xt2, scalar1=16.0, scalar2=TWO23,
                            op0=AL.mult, op1=AL.add)
    t2 = sb.tile([T, BF], f32, tag="t2")
    nc.vector.tensor_single_scalar(out=t2[:], in_=t1[:], scalar=TWO23, op=AL.subtract)
    t3 = sb.tile([T, BF], f32, tag="t3")
    nc.vector.scalar_tensor_tensor(out=t3[:], in0=t2[:], scalar=0.0625, in1=xt2,
                                   op0=AL.mult, op1=AL.is_gt)
    ub = sb.tile([T, BF], bf16, tag="ub")
    nc.vector.tensor_tensor(out=ub[:], in0=t2[:], in1=t3[:], op=AL.subtract)

    # ---- one-hot level masks --------------------------------------------
    # level q <=> u == q+1 for q in 1..14; q0 <=> u<=1 minus the x==0 case;
    # q15 <=> x == 0. Plain equality masks first (they gate the matmuls),
    # zero-mask and fused q0 afterwards.
    oha = sb.tile([T, Q, BF], f8, tag="oha")
    for q in range(1, Q - 1):
        eng = nc.gpsimd if q >= Q - 1 - N_POOL_CMP else nc.vector
        eng.tensor_single_scalar(out=oha[:, q, :], in_=ub[:], scalar=float(q + 1),
                                 op=AL.is_equal)
    nc.vector.tensor_single_scalar(out=oha[:, Q - 1, :], in_=xt2, scalar=0.0,
                                   op=AL.is_equal)
    nc.vector.scalar_tensor_tensor(out=oha[:, 0, :], in0=ub[:], scalar=1.0,
                                   in1=oha[:, Q - 1, :],
                                   op0=AL.is_le, op1=AL.subtract)

    # ---- matmul chains ---------------------------------------------------
    # DoubleRow fp8 passes: two K-chunks per matmul. Pair order puts the
    # plain equality masks first, the q0/q15 pair (which needs the zero mask)
    # last, then the id-window band pair.
    pairs = [(2, 3), (4, 5), (6, 7), (8, 9), (10, 11), (12, 13), (14, 15), (0, 1)]
    DR = mybir.MatmulPerfMode.DoubleRow
    accs = []
    for mc in range(2):
        p = psp.tile([128, BF], f32, tag=f"acc{mc}")
        for ci, (qa, qb) in enumerate(pairs):
            assert qb == qa + 1
            nc.tensor.matmul(p[:], sla[:, qa:qb + 1, mc * 128:(mc + 1) * 128],
                             oha[:, qa:qb + 1, :],
                             start=(ci == 0), stop=False, perf_mode=DR)
        j_lo = 0 if mc == 0 else 1
        nc.tensor.matmul(p[:], tri[:], idr[:, j_lo:j_lo + 2],
                         start=False, stop=True, perf_mode=DR)
        accs.append(p)

    # ---- parity, grouped reduce over f, threshold ------------------------
    fin = sb.tile([128, 2 * B], f32, tag="fin")
    for mc in range(2):
        if PARITY_MODE == "mod":
            seq = sb.tile([128, BF], bf16, tag=f"seq{mc}")
            nc.vector.tensor_single_scalar(out=seq[:], in_=accs[mc][:], scalar=2.0,
                                           op=AL.mod)
            thr = 20.5
        else:
            si = sb.tile([128, BF], i32, tag=f"si{mc}")
            nc.vector.tensor_copy(out=si[:], in_=accs[mc][:])
            seq = sb.tile([128, BF], i32, tag=f"seq{mc}")
            nc.vector.tensor_single_scalar(out=seq[:], in_=si[:], scalar=1,
                                           op=AL.bitwise_and)
            thr = 20
        red = sb.tile([128, B], seq[:].dtype, tag=f"red{mc}")
        with nc.allow_low_precision(reason="exact small-int accumulation (<=40)"):
            nc.vector.tensor_reduce(out=red[:], in_=seq[:].rearrange("p (b f) -> p b f", b=B),
                                    axis=mybir.AxisListType.X, op=AL.add)
        cmpo = sb.tile([128, B], f32, tag=f"cmp{mc}")
        nc.vector.tensor_single_scalar(out=cmpo[:], in_=red[:], scalar=thr, op=AL.is_gt)
        nc.vector.tensor_scalar(out=fin[:, mc * B:(mc + 1) * B], in0=cmpo[:],
                                scalar1=2.0, scalar2=-1.0, op0=AL.mult, op1=AL.add)

    # ---- transpose [128, 16] -> [16, 128] on PE, DMA straight from PSUM --
    pst = psp.tile([2 * B, 128], f32, tag="pst")
    nc.tensor.transpose(out=pst[:], in_=fin[:], identity=iden[:])
    so = sb.tile([2 * B, 128], f32, tag="so")
    nc.vector.tensor_copy(out=so[:], in_=pst[:])
    nc.sync.dma_start(out=out_d[:, 0:128], in_=so[0:B, :])
    nc.gpsimd.dma_start(out=out_d[:, 128:256], in_=so[B:2 * B, :])


def build_nc():
    nc = bacc.Bacc("TRN2", target_bir_lowering=False, debug=False)
    xt_d = nc.dram_tensor("xt", [T, B, F], f32, kind="ExternalInput")
    lwb_d = nc.dram_tensor("lwb", [Q, W], f8, kind="ExternalInput")
    idt_d = nc.dram_tensor("idt", [W, F], f8, kind="ExternalInput")
    out_d = nc.dram_tensor("out", [B, DS], f32, kind="ExternalOutput")
    with tile.TileContext(nc) as tc:
        with ExitStack() as ctx:
            emit_kernel(nc, tc, ctx, xt_d[:], lwb_d[:], idt_d[:], out_d[:])
    nc.compile()
    return nc


def make_in_maps(x, level_hvs, id_hvs):
    x = np.asarray(x, dtype=np.float32)
    L = np.asarray(level_hvs, dtype=np.int32)
    ID = np.asarray(id_hvs, dtype=np.int32)
    # time-reverse + transpose to [T, B, F] (layout only)
    xt = np.ascontiguousarray(x[:, ::-1, :].transpose(1, 0, 2))
    LL2 = np.concatenate([L, L], axis=1).astype(ml_dtypes.float8_e4m3)
    II2 = np.concatenate([ID, ID], axis=1).astype(ml_dtypes.float8_e4m3)
    in_maps = []
    for c in range(NCORE):
        d0 = c * DS
        s = (d0 - 127) % D
        lwb_c = np.ascontiguousarray(LL2[:, s:s + W])
        s2 = (d0 - 128) % D
        idt_c = np.ascontiguousarray(II2[:, s2:s2 + W].T)
        in_maps.append({"xt": xt, "lwb": lwb_c, "idt": idt_c})
    return in_maps


_NC_CACHE = {}


def kernel(x, level_hvs, id_hvs):
    if "nc" not in _NC_CACHE:
        _NC_CACHE["nc"] = build_nc()
    nc = _NC_CACHE["nc"]
    in_maps = make_in_maps(x, level_hvs, id_hvs)
    res = run_bass_kernel_spmd(nc, in_maps, list(range(NCORE)))
    full = np.empty((B, D), dtype=np.float32)
    for c in range(NCORE):
        full[:, c * DS:(c + 1) * DS] = res.results[c]["out"]
    return full


# revision 15
# speedup vs baseline: 1.6013x; 1.0259x over previous
"""Trainium2 Bass kernel for nn_Encoder_61753039782402 (HD-computing encoder).

Math: out[b,d] = sign( sum_f parity( sum_t L[q(b,t,f), d-t] + sum_t id[f, d-t] ) - 20.5 )
where q(b,t,f) = trunc(16*x[b,t,f] - 1) wrapped mod 16 (x==0 -> 15).

Implementation per core (D sharded 8 ways, 256 output columns each):
  - one-hot level masks OH_q[t,(b,f)] built via an exact floor trick; the 15
    equality compares are split across DVE and GPSIMD
  - shifted-L "circulant band" tiles SL_q[u,d'] = L[q, d0+d'-127+u] gathered by
    two overlapping strided DMAs from the bf16 L window input (time axis
    reversed so all strides are positive; x is passed time-reversed)
  - PSUM-accumulated bf16 matmul chain: 16 level chunks + 2 triangular-constant
    chunks folding in the id window-sum
  - parity (mod 2) + grouped reduce over f + threshold on DVE; PE transpose;
    direct DMA out of PSUM
Host-side prep is layout/dtype only: slicing the doubled tables per core,
time-reversing/transposing x, int->bf16 casts.
"""

from contextlib import ExitStack

import numpy as np
import ml_dtypes

import concourse.bass as bass
import concourse.bacc as bacc
import concourse.mybir as mybir
import concourse.tile as tile
from concourse.bass_utils import run_bass_kernel_spmd

B, T, F, Q, D = 8, 128, 40, 16, 2048
NCORE = 8
DS = D // NCORE  # 256 output columns per core
W = 384          # per-core window-slice width for lwb / idt
BF = B * F       # 320
f32, bf16, i32 = mybir.dt.float32, mybir.dt.bfloat16, mybir.dt.int32
f8 = mybir.dt.float8e4
AL = mybir.AluOpType
TWO23 = float(2 ** 23)

PARITY_MODE = "int"   # "mod" (single fused fp-mod op) fails walrus tensor_scalar_valid_ops
N_POOL_CMP = 7         # of the 14 plain equality compares, how many go to GPSIMD


def emit_kernel(nc, tc, ctx, xt_d, lwb_d, idt_d, out_d):
    sb = ctx.enter_context(tc.tile_pool(name="sb", bufs=1))
    psp = ctx.enter_context(tc.tile_pool(name="psp", bufs=1, space=bass.MemorySpace.PSUM))

    # ---- input DMAs ------------------------------------------------------
    xt = sb.tile([T, B, F], f32, tag="xt")
    nc.sync.dma_start(out=xt[:], in_=xt_d)
    xt2 = xt[:].rearrange("u b f -> u (b f)")  # [128, 320]

    # shifted-L gathers: sl[u, q, d'] = Lw[q, u + d']   (overlapping reads)
    # issued from ACT's HWDGE path to overlap with SP's x trigger
    sla = sb.tile([128, Q, DS], f8, tag="sla")
    for g in range(4):
        src = bass.AP(tensor=lwb_d.tensor, offset=g * 4 * W,
                      ap=[[1, 128], [W, 4], [1, DS]])
        nc.scalar.dma_start(out=sla[:, g * 4:(g + 1) * 4, :], in_=src)

    # id window slice, transposed [src, f]: one 3-chunk DMA via Pool SWDGE
    idb = sb.tile([128, 3, F], f8, tag="idb")
    nc.gpsimd.dma_start(out=idb[:], in_=idt_d.rearrange("(j p) f -> p j f", p=128))

    # ---- GPSIMD-side constants ------------------------------------------
    iot = sb.tile([128, 128], i32, tag="iot")
    nc.gpsimd.iota(out=iot[:], pattern=[[-1, 128]], base=0, channel_multiplier=1)  # p - m
    tri = sb.tile([128, 2, 128], f8, tag="tri")
    nc.gpsimd.tensor_single_scalar(out=tri[:, 0, :], in_=iot[:], scalar=0, op=AL.is_gt)  # m < p
    nc.gpsimd.tensor_single_scalar(out=tri[:, 1, :], in_=iot[:], scalar=0, op=AL.is_le)  # m >= p
    iden = sb.tile([128, 128], f32, tag="iden")
    nc.gpsimd.tensor_single_scalar(out=iden[:], in_=iot[:], scalar=0, op=AL.is_equal)

    # replicate id window over b (log-doubling) on GPSIMD
    idr = sb.tile([128, 3, B, F], f8, tag="idr")
    nc.gpsimd.tensor_copy(out=idr[:, :, 0, :], in_=idb[:])
    nc.gpsimd.tensor_copy(out=idr[:, :, 1:2, :], in_=idr[:, :, 0:1, :])
    nc.gpsimd.tensor_copy(out=idr[:, :, 2:4, :], in_=idr[:, :, 0:2, :])
    nc.gpsimd.tensor_copy(out=idr[:, :, 4:8, :], in_=idr[:, :, 0:4, :])

    # ---- exact floor(16x) via round-to-nearest + fixup (DVE) ------------
    t1 = sb.tile([T, BF], f32, tag="t1")
    nc.vector.tensor_scalar(out=t1[:], in0=xt2, scalar1=16.0, scalar2=TWO23,
                            op0=AL.mult, op1=AL.add)
    t2 = sb.tile([T, BF], f32, tag="t2")
    nc.vector.tensor_single_scalar(out=t2[:], in_=t1[:], scalar=TWO23, op=AL.subtract)
    t3 = sb.tile([T, BF], f32, tag="t3")
    nc.vector.scalar_tensor_tensor(out=t3[:], in0=t2[:], scalar=0.0625, in1=xt2,
                                   op0=AL.mult, op1=AL.is_gt)
    ub = sb.tile([T, BF], bf16, tag="ub")
    nc.vector.tensor_tensor(out=ub[:], in0=t2[:], in1=t3[:], op=AL.subtract)

    # ---- one-hot level masks --------------------------------------------
    # level q <=> u == q+1 for q in 1..14; q0 <=> u<=1 minus the x==0 case;
    # q15 <=> x == 0. Plain equality masks first (they gate the matmuls),
    # zero-mask and fused q0 afterwards.
    oha = sb.tile([T, Q, BF], f8, tag="oha")
    for q in [2, 3, 4, 5, 6, 7, 1] + list(range(8, Q - 1)):
        eng = nc.gpsimd if q >= Q - 1 - N_POOL_CMP else nc.vector
        eng.tensor_single_scalar(out=oha[:, q, :], in_=ub[:], scalar=float(q + 1),
                                 op=AL.is_equal)
    nc.vector.tensor_single_scalar(out=oha[:, Q - 1, :], in_=xt2, scalar=0.0,
                                   op=AL.is_equal)
    nc.vector.scalar_tensor_tensor(out=oha[:, 0, :], in0=ub[:], scalar=1.0,
                                   in1=oha[:, Q - 1, :],
                                   op0=AL.is_le, op1=AL.subtract)

    # ---- matmul chains ---------------------------------------------------
    # DoubleRow fp8 passes: two K-chunks per matmul. Pair order puts the
    # plain equality masks first, the q0/q15 pair (which needs the zero mask)
    # last, then the id-window band pair.
    pairs = [(2, 3), (4, 5), (6, 7), (8, 9), (10, 11), (12, 13), (14, 15), (0, 1)]
    DR = mybir.MatmulPerfMode.DoubleRow
    accs = []
    for mc in range(2):
        p = psp.tile([128, BF], f32, tag=f"acc{mc}")
        for ci, (qa, qb) in enumerate(pairs):
            assert qb == qa + 1
            nc.tensor.matmul(p[:], sla[:, qa:qb + 1, mc * 128:(mc + 1) * 128],
                             oha[:, qa:qb + 1, :],
                             start=(ci == 0), stop=False, perf_mode=DR)
        j_lo = 0 if mc == 0 else 1
        nc.tensor.matmul(p[:], tri[:], idr[:, j_lo:j_lo + 2],
                         start=False, stop=True, perf_mode=DR)
        accs.append(p)

    # ---- parity, grouped reduce over f, threshold ------------------------
    fin = sb.tile([128, 2 * B], f32, tag="fin")
    for mc in range(2):
        if PARITY_MODE == "mod":
            seq = sb.tile([128, BF], bf16, tag=f"seq{mc}")
            nc.vector.tensor_single_scalar(out=seq[:], in_=accs[mc][:], scalar=2.0,
                                           op=AL.mod)
            thr = 20.5
        else:
            si = sb.tile([128, BF], i32, tag=f"si{mc}")
            nc.scalar.copy(out=si[:], in_=accs[mc][:])
            seq = sb.tile([128, BF], i32, tag=f"seq{mc}")
            nc.vector.tensor_single_scalar(out=seq[:], in_=si[:], scalar=1,
                                           op=AL.bitwise_and)
            thr = 20
        red = sb.tile([128, B], seq[:].dtype, tag=f"red{mc}")
        with nc.allow_low_precision(reason="exact small-int accumulation (<=40)"):
            nc.vector.tensor_reduce(out=red[:], in_=seq[:].rearrange("p (b f) -> p b f", b=B),
                                    axis=mybir.AxisListType.X, op=AL.add)
        cmpo = sb.tile([128, B], f32, tag=f"cmp{mc}")
        nc.vector.tensor_single_scalar(out=cmpo[:], in_=red[:], scalar=thr, op=AL.is_gt)
        nc.vector.tensor_scalar(out=fin[:, mc * B:(mc + 1) * B], in0=cmpo[:],
                                scalar1=2.0, scalar2=-1.0, op0=AL.mult, op1=AL.add)

    # ---- transpose [128, 16] -> [16, 128] on PE, DMA straight from PSUM --
    pst = psp.tile([2 * B, 128], f32, tag="pst")
    nc.tensor.transpose(out=pst[:], in_=fin[:], identity=iden[:])
    so = sb.tile([2 * B, 128], f32, tag="so")
    nc.scalar.copy(out=so[:], in_=pst[:])
    nc.sync.dma_start(out=out_d[:, 0:128], in_=so[0:B, :])
    nc.gpsimd.dma_start(out=out_d[:, 128:256], in_=so[B:2 * B, :])


def build_nc():
    nc = bacc.Bacc("TRN2", target_bir_lowering=False, debug=False)
    xt_d = nc.dram_tensor("xt", [T, B, F], f32, kind="ExternalInput")
    lwb_d = nc.dram_tensor("lwb", [Q, W], f8, kind="ExternalInput")
    idt_d = nc.dram_tensor("idt", [W, F], f8, kind="ExternalInput")
    out_d = nc.dram_tensor("out", [B, DS], f32, kind="ExternalOutput")
    with tile.TileContext(nc) as tc:
        with ExitStack() as ctx:
            emit_kernel(nc, tc, ctx, xt_d[:], lwb_d[:], idt_d[:], out_d[:])
    nc.compile()
    return nc


def make_in_maps(x, level_hvs, id_hvs):
    x = np.asarray(x, dtype=np.float32)
    L = np.asarray(level_hvs, dtype=np.int32)
    ID = np.asarray(id_hvs, dtype=np.int32)
    # time-reverse + transpose to [T, B, F] (layout only)
    xt = np.ascontiguousarray(x[:, ::-1, :].transpose(1, 0, 2))
    LL2 = np.concatenate([L, L], axis=1).astype(ml_dtypes.float8_e4m3)
    II2 = np.concatenate([ID, ID], axis=1).astype(ml_dtypes.float8_e4m3)
    in_maps = []
    for c in range(NCORE):
        d0 = c * DS
        s = (d0 - 127) % D
        lwb_c = np.ascontiguousarray(LL2[:, s:s + W])
        s2 = (d0 - 128) % D
        idt_c = np.ascontiguousarray(II2[:, s2:s2 + W].T)
        in_maps.append({"xt": xt, "lwb": lwb_c, "idt": idt_c})
    return in_maps


_NC_CACHE = {}


def kernel(x, level_hvs, id_hvs):
    if "nc" not in _NC_CACHE:
        _NC_CACHE["nc"] = build_nc()
    nc = _NC_CACHE["nc"]
    in_maps = make_in_maps(x, level_hvs, id_hvs)
    res = run_bass_kernel_spmd(nc, in_maps, list(range(NCORE)))
    full = np.empty((B, D), dtype=np.float32)
    for c in range(NCORE):
        full[:, c * DS:(c + 1) * DS] = res.results[c]["out"]
    return full
